# revision 2
# baseline (speedup 1.0000x reference)
"""AttnReadout kernel for Trainium2 (8 NeuronCores, data-parallel over batch).

Computes, for x:[B,N,D], last_nodes:[B], W_u/W_v:[D,D], b_u:[D], W_e:[D,1]:
    x_u   = x @ W_u + b_u
    x_v   = x[b, last_nodes[b]] @ W_v
    e     = sigmoid(x_u + x_v[:,None]) @ W_e
    alpha = softmax(e, axis=-2)
    out   = sum(x * alpha, axis=-2)          # [B, D]

Sharding: batch is split 8 ways (32 batches per core); the tiny weights are
replicated. No cross-core communication.
"""

import numpy as np
import ml_dtypes
from contextlib import ExitStack

try:
    import concourse.bass as bass
except ImportError:  # stock container: repo lives in /opt
    import sys

    sys.path.insert(0, "/opt/trn_rl_repo")
    import concourse.bass as bass

from concourse import bacc, mybir
import concourse.tile as tile
from concourse.bass_utils import run_bass_kernel_spmd

DT = mybir.dt
BF16 = DT.bfloat16
F32 = DT.float32
F8 = DT.float8e4
DoubleRow = mybir.MatmulPerfMode.DoubleRow

B, N, D = 256, 512, 256
NCORES = 8
BSH = B // NCORES  # 32 batches per core
P = 128
NCH_N = N // P  # 4 chunks of the node axis
NCH_D = D // P  # 2 chunks of the feature axis
GRP = 8  # softmax batching group

Sigmoid = mybir.ActivationFunctionType.Sigmoid
Exp = mybir.ActivationFunctionType.Exp


def build_dma_nc(nb=BSH, reps=1, lgrp=2, dma_eng="gpsimd", dma_dt="f8", grp=GRP):
    """x-load path in isolation: group loads + token consumer + out store."""
    nc = bacc.Bacc("TRN2", target_bir_lowering=False, debug=False, num_devices=NCORES)
    ddt = {"f8": F8, "bf16": BF16, "f32": F32}[dma_dt]
    x_d = nc.dram_tensor("x", [nb, N, D], F32, kind="ExternalInput")
    out_d = nc.dram_tensor("out", [nb, D], F32, kind="ExternalOutput")
    with tile.TileContext(nc) as tc, ExitStack() as ctx:
        consts = ctx.enter_context(tc.tile_pool(name="consts", bufs=1))
        dpool = ctx.enter_context(tc.tile_pool(name="dob", bufs=3))
        zo = consts.tile([nb, D], F32, tag="zo")
        nc.vector.memset(zo[:], 0.0)
        eng = getattr(nc, dma_eng)
        for r in range(reps):
            for g0 in range(0, nb, grp):
                xg = dpool.tile([P, grp, NCH_N, D], ddt, tag="xg")
                for b0 in range(0, grp, lgrp):
                    eng.dma_start(
                        xg[:, b0 : b0 + lgrp],
                        x_d.ap()[g0 + b0 : g0 + b0 + lgrp].rearrange(
                            "b (a p) d -> p b a d", p=P
                        ),
                    )
                nc.vector.tensor_copy(
                    zo[0:1, (g0 // grp) * 16 : (g0 // grp) * 16 + 16],
                    xg[0:1, 0, 0, 0:16],
                )
            nc.sync.dma_start(out_d.ap()[:], zo[:])
    nc.compile()
    return nc


def build_nc(nb=BSH, reps=1, grp=None, pipelined=True, skip_tr=False, skip_cp=False, inter=False, xu_bufs=3, tr_bufs=2, lgrp=2, pool8=True):
    GRP = grp or globals()["GRP"]
    nc = bacc.Bacc("TRN2", target_bir_lowering=False, debug=False, num_devices=NCORES)

    x_d = nc.dram_tensor("x", [nb, N, D], F32, kind="ExternalInput")
    offs_d = nc.dram_tensor("offs", [nb, 1], DT.int32, kind="ExternalInput")
    wu_d = nc.dram_tensor("wu", [P, NCH_D, D], F8, kind="ExternalInput")
    wv_d = nc.dram_tensor("wv", [P, NCH_D, D], BF16, kind="ExternalInput")
    we_d = nc.dram_tensor("we", [P, NCH_D, 32], F8, kind="ExternalInput")
    bu_d = nc.dram_tensor("bu", [P, NCH_D], F32, kind="ExternalInput")
    idf_d = nc.dram_tensor("idf", [P, P], F32, kind="ExternalInput")
    idb_d = nc.dram_tensor("idb", [P, P], BF16, kind="ExternalInput")
    out_d = nc.dram_tensor("out", [nb, D], F32, kind="ExternalOutput")

    with tile.TileContext(nc) as tc, ExitStack() as ctx:
        consts = ctx.enter_context(tc.tile_pool(name="consts", bufs=1))
        # one group-sized x tile per buffer: the whole group loads in a single
        # SWDGE DMA (amortizes the ~1us Q7 descriptor-gen fixed cost 8x)
        xnat = ctx.enter_context(tc.tile_pool(name="xnat", bufs=3))
        xtp = ctx.enter_context(tc.tile_pool(name="xt", bufs=6))
        spool = ctx.enter_context(tc.tile_pool(name="s", bufs=3))
        smallp = ctx.enter_context(tc.tile_pool(name="small", bufs=2))
        # PSUM budget: 3 + 2 + 2 + 1 = 8 banks (ptr pool gone with the XBAR
        # transposes; pe/pout double-buffered so e-quads and pooling pipeline).
        pxu = ctx.enter_context(tc.tile_pool(name="pxu", bufs=xu_bufs, space="PSUM"))
        pe_p = ctx.enter_context(tc.tile_pool(name="pe", bufs=2, space="PSUM"))
        pout = ctx.enter_context(tc.tile_pool(name="pout", bufs=2, space="PSUM"))
        psm = ctx.enter_context(tc.tile_pool(name="psm", bufs=1, space="PSUM"))

        # ---- constants ----
        wu_sb = consts.tile([P, NCH_D, D], F8, tag="wu")
        nc.sync.dma_start(wu_sb[:], wu_d.ap())
        wv_sb = consts.tile([P, NCH_D, D], BF16, tag="wv")
        nc.sync.dma_start(wv_sb[:], wv_d.ap())
        we_sb = consts.tile([P, NCH_D, 32], F8, tag="we")
        nc.sync.dma_start(we_sb[:], we_d.ap())
        bu_sb = consts.tile([P, NCH_D], F32, tag="bu")
        nc.sync.dma_start(bu_sb[:], bu_d.ap())
        idf_sb = consts.tile([P, P], F32, tag="idf")
        nc.sync.dma_start(idf_sb[:], idf_d.ap())
        idb_sb = consts.tile([P, P], BF16, tag="idb")
        nc.sync.dma_start(idb_sb[:], idb_d.ap())
        # fp8 identity for the x transposes (transpose-mode keeps dtype)
        id8_sb = consts.tile([P, P], F8, tag="id8")
        nc.vector.tensor_copy(id8_sb[:], idb_sb[:])
        offs_sb = consts.tile([nb, 1], DT.int32, tag="offs")
        nc.sync.dma_start(offs_sb[:], offs_d.ap())

        # ---- phase 0: gather x_last, compute xvb = W_v^T x_last + b_u ----
        xlast = consts.tile([nb, D], F32, tag="xlast")
        nc.gpsimd.indirect_dma_start(
            out=xlast[:],
            out_offset=None,
            in_=x_d.ap().rearrange("b n d -> (b n) d"),
            in_offset=bass.IndirectOffsetOnAxis(ap=offs_sb[:, :1], axis=0),
        )
        # transpose to [D, nb] (d-major) so the W_v matmul can contract over d
        xlt_ps = psm.tile([P, NCH_D, nb], F32, tag="sm")
        for c in range(NCH_D):
            nc.tensor.matmul(
                xlt_ps[:, c, :], xlast[:, c * P : (c + 1) * P], idf_sb[:nb, :nb]
            )
        xlt = consts.tile([P, NCH_D, nb], BF16, tag="xlt")
        nc.vector.tensor_copy(xlt[:], xlt_ps[:])

        xt_const = None
        if skip_tr or skip_cp:  # timing experiments
            xt_const = consts.tile([P, 2 * N], F8, tag="xtc")
            nc.vector.memset(xt_const[:], 0.001)

        xvb = consts.tile([P, NCH_D, nb], F32, tag="xvb")
        for j in range(NCH_D):
            xv_ps = psm.tile([P, nb], F32, tag="sm")
            for c in range(NCH_D):
                nc.tensor.matmul(
                    xv_ps[:],
                    wv_sb[:, c, j * P : (j + 1) * P],
                    xlt[:, c, :],
                    start=(c == 0),
                    stop=(c == NCH_D - 1),
                )
            nc.vector.tensor_copy(xvb[:, j, :], xv_ps[:])
            nc.vector.tensor_scalar_add(xvb[:, j, :], xvb[:, j, :], bu_sb[:, j : j + 1])

        # ---- main loop ----
        # The "final" phase (softmax + weighted pooling) of each group is
        # emitted one group late: its softmax/p-transpose prologue goes in
        # front of the next group's batch loop, and its per-batch pooling
        # matvecs are interleaved between the next group's e-matvecs on
        # DISJOINT PE column groups, so the hardware runs them concurrently.

        def final_prologue(g, gb, e_all):
            # softmax over n (batched across the group)
            m_t = smallp.tile([gb, 1], F32, tag="mx")
            nc.vector.tensor_reduce(
                m_t[:], e_all[:], axis=mybir.AxisListType.X, op=mybir.AluOpType.max
            )
            mn_t = smallp.tile([gb, 1], F32, tag="mn")
            nc.vector.tensor_scalar_mul(mn_t[:], m_t[:], -1.0)
            p_t = smallp.tile([gb, N], BF16, tag="p")
            sum_t = smallp.tile([gb, 1], F32, tag="sum")
            nc.scalar.activation(
                p_t[:], e_all[:], Exp, bias=mn_t[:, :1], accum_out=sum_t[:, :1]
            )
            r_t = smallp.tile([gb, 1], F32, tag="r")
            nc.vector.reciprocal(r_t[:], sum_t[:])
            # normalize alpha BEFORE the transpose so the pooling output is
            # final — o_sb rows can then store straight to DRAM (no SBUF->SBUF
            # gather DMAs, which would serialize against the XBAR transposes)
            nc.vector.tensor_scalar_mul(p_t[:], p_t[:], r_t[:, :1])

            # transpose p to n-major for use as matmul weights
            pt_ps = psm.tile([P, NCH_N, gb], F32, tag="sm")
            for i in range(NCH_N):
                nc.tensor.matmul(
                    pt_ps[:, i, :], p_t[:, i * P : (i + 1) * P], idb_sb[:gb, :gb]
                )
            # gb real columns + 32 zero columns so each batch's lhsT slab
            # [bb : bb+32] is in-bounds (col 0 real, rest harmless)
            pt_sb = smallp.tile([P, NCH_N, gb + 32], BF16, tag="pt")
            nc.vector.memset(pt_sb[:, :, gb : gb + 32], 0.0)
            nc.vector.tensor_copy(pt_sb[:, :, 0:gb], pt_ps[:])
            return {"g": g, "gb": gb, "pt": pt_sb,
                    "o_ps": None, "xbs": None}

        def final_mms(ctx, bb, js):
            # pooling matvecs for prev-group batch bb, n-chunks in `js`;
            # column group (bb+2)%4 — disjoint from the current e-matvec's
            qo = (bb + 2) % 4
            xg_, bi = ctx["xbs"][bb]
            for i in js:
                nc.tensor.matmul(
                    ctx["o_ps"][qo * 32 : qo * 32 + 32, :],
                    ctx["pt"][:, i, bb : bb + 32],
                    xg_[:, bi, i, :],
                    start=(i == 0),
                    stop=(i == NCH_N - 1),
                    tile_position=(0, qo * 32),
                )

        def final_drain(ctx, bb):
            # after each quad: PSUM -> SBUF, then store the (already
            # normalized) rows straight to DRAM; row base+m sits at
            # partition 32*((m+2)%4)
            o_sb = spool.tile([P, D], F32, tag="osb")
            nc.vector.tensor_copy(o_sb[:], ctx["o_ps"][:])
            base = bb - (bb % 4)
            g0 = (ctx["g"] % nb_grp) * GRP
            nc.scalar.dma_start(
                out_d.ap()[g0 + base : g0 + base + 2, :], o_sb[64 : 97 : 32, :]
            )
            nc.scalar.dma_start(
                out_d.ap()[g0 + base + 2 : g0 + base + 4, :], o_sb[0 : 33 : 32, :]
            )

        def final_epilogue(ctx):
            pass

        def compute_group(g, prev):
            gb = min(GRP, nb - (g % nb_grp) * GRP)
            assert gb % 4 == 0
            ctx = None
            if prev is not None:
                pg, pgb, pe_all, pxbs = prev
                ctx = final_prologue(pg, pgb, pe_all)
                ctx["xbs"] = pxbs
            e_all = smallp.tile([gb, N], F32, tag="eall")
            g0 = (g % nb_grp) * GRP
            # load the group's x in gb/lgrp SWDGE DMAs (f32 -> fp8/bf16 cast),
            # natural layout per batch: xg[p, bb, a, d] = x[g0+bb, a*128+p, d].
            # lgrp batches per DMA amortizes the ~1us Q7 descriptor-gen fixed
            # cost while keeping completion granularity fine enough to feed
            # the first transposes early.
            xg = xnat.tile([P, gb, NCH_N, D], F8, tag="xg")
            for b0 in range(0, gb, lgrp):
                b1 = min(b0 + lgrp, gb)
                nc.gpsimd.dma_start(
                    xg[:, b0:b1],
                    x_d.ap()[g0 + b0 : g0 + b1].rearrange(
                        "b (a p) d -> p b a d", p=P
                    ),
                )
            xbs = []
            e_ps = None
            for bb in range(gb):
                b = g0 + bb
                xbs.append((xg, bb))

                # transpose to d-major XT [d, n] via PE (transpose-mode,
                # bf16 PSUM). All 8 blocks land in ONE full-bank PSUM tile,
                # drained by ONE bit-copy per batch (alternating ACT/DVE) —
                # per-block drains ping-pong with PE on bank hazards.
                if skip_tr:
                    xt = xt_const
                else:
                    # PAIR transpose via the XBAR DMA (SBUF->SBUF): view fp8 x
                    # as bf16 pairs [128, 512]; the 16-bit transpose lands at
                    # xt fp8 offset 2k+t for (d=2i+t, n=k) — exactly the
                    # pair-interleaved layout the DoubleRow rhs AP consumes.
                    # No PE time, no PSUM, no drain copy.
                    xt = xtp.tile([P, 2 * N], F8, tag="xt")
                    nc.sync.dma_start(
                        xt[:].bitcast(BF16).rearrange("p (a q) -> p a q", q=P),
                        xg[:, bb].bitcast(BF16),
                        transpose=True,
                    )

                # x_u^T = W_u^T @ XT: fp8 DoubleRow packs the full d=256
                # contraction into one matmul per output chunk j. Wu is
                # host-paired (d = 2*Ki + Ko) to match xt's pair interleave.
                s_t = spool.tile([P, NCH_D, N], F8, tag="s")
                for j in range(NCH_D):
                    xu = pxu.tile([P, N], F32, tag="xu")
                    nc.tensor.matmul(
                        xu[:],
                        wu_sb[:, :, j * P : (j + 1) * P],
                        xt[:].rearrange("p (k t) -> p t k", t=2),
                        perf_mode=DoubleRow,
                    )
                    nc.scalar.activation(
                        s_t[:, j, :], xu[:], Sigmoid, bias=xvb[:, j, b : b + 1]
                    )

                # e[n] = W_e^T @ S : one fp8 DoubleRow matvec (M=32 zero-pad
                # trick unchanged); lands on psum partition 32*(bb%4).
                # Interleaved with the previous group's pooling matvecs
                # (disjoint column groups -> concurrent on the PE array).
                q = bb % 4
                if q == 0:
                    e_ps = pe_p.tile([P, N], F32, tag="e")
                    if ctx is not None and bb < ctx["gb"]:
                        ctx["o_ps"] = pout.tile([P, D], F32, name="o_ps", tag="o")
                # (plain fp8 matmuls here: DoubleRow + col tile_position is
                # illegal — XBUS budget)
                for j in range(NCH_D):
                    nc.tensor.matmul(
                        e_ps[q * 32 : q * 32 + 32, :],
                        we_sb[:, j, :],
                        s_t[:, j, :],
                        start=(j == 0),
                        stop=(j == NCH_D - 1),
                        tile_position=(0, q * 32),
                    )
                    if ctx is not None and bb < ctx["gb"]:
                        final_mms(ctx, bb, (2 * j, 2 * j + 1))
                if ctx is not None and bb < ctx["gb"] and (q == 3 or bb == ctx["gb"] - 1):
                    final_drain(ctx, bb)
                if q == 3 or bb == gb - 1:
                    # engines can't address strided partitions, but DMA can:
                    # PSUM -> SBUF copy (contiguous), then SBUF->SBUF DMA gather
                    e_sb = spool.tile([P, N], F32, tag="esb")
                    nc.vector.tensor_copy(e_sb[:], e_ps[:])
                    nc.scalar.dma_start(
                        e_all[bb - q : bb + 1, :], e_sb[0 : 32 * q + 1 : 32, :]
                    )
            if ctx is not None:
                for bb in range(gb, ctx["gb"]):  # leftover when prev group bigger
                    q = bb % 4
                    if q == 0:
                        ctx["o_ps"] = pout.tile([P, D], F32, name="o_ps", tag="o")
                    final_mms(ctx, bb, range(NCH_N))
                    if q == 3 or bb == ctx["gb"] - 1:
                        final_drain(ctx, bb)
                final_epilogue(ctx)
            return gb, e_all, xbs

        def final_tail(prev):
            # the last group's final phase has no next group to hide in
            pg, pgb, pe_all, pxbs = prev
            ctx = final_prologue(pg, pgb, pe_all)
            ctx["xbs"] = pxbs
            for bb in range(pgb):
                q = bb % 4
                if q == 0:
                    ctx["o_ps"] = pout.tile([P, D], F32, name="o_ps", tag="o")
                final_mms(ctx, bb, range(NCH_N))
                if q == 3 or bb == pgb - 1:
                    final_drain(ctx, bb)
            final_epilogue(ctx)

        # reps>1 repeats the whole sweep (for slope-based timing)
        nb_grp = (nb + GRP - 1) // GRP
        pending = None
        for g in range(nb_grp * reps):
            if inter:
                pending = (g,) + compute_group(g, pending)
            else:
                done = (g,) + compute_group(g, None)
                if pending is not None:
                    final_tail(pending)
                pending = done
        final_tail(pending)

    nc.compile()
    return nc


_NC_CACHE = {}


def _get_nc(nb=BSH, reps=1):
    if (nb, reps) not in _NC_CACHE:
        _NC_CACHE[(nb, reps)] = build_nc(nb, reps)
    return _NC_CACHE[(nb, reps)]


def make_in_maps(x, last_nodes, W_u, b_u, W_v, W_e, ncores=NCORES):
    x = np.asarray(x, dtype=np.float32)
    last_nodes = np.asarray(last_nodes).astype(np.int64)
    W_u = np.asarray(W_u, dtype=np.float32)
    b_u = np.asarray(b_u, dtype=np.float32)
    W_v = np.asarray(W_v, dtype=np.float32)
    W_e = np.asarray(W_e, dtype=np.float32)

    nb = x.shape[0] // ncores
    xs = x.reshape(ncores, nb, N, D)
    ln = last_nodes.reshape(ncores, nb)
    offs = (np.arange(nb)[None, :] * N + ln).astype(np.int32).reshape(ncores, nb, 1)

    bf = ml_dtypes.bfloat16
    f8 = ml_dtypes.float8_e4m3
    # DoubleRow pairing: wu_h[i, t, m] = W_u[2i+t, m] matches the
    # pair-interleaved x^T layout from the bf16-view transposes.
    wu_h = np.ascontiguousarray(W_u.reshape(P, 2, D)).astype(f8)
    wv_h = np.ascontiguousarray(
        W_v.reshape(NCH_D, P, D).transpose(1, 0, 2)
    ).astype(bf)
    we_h = np.zeros((P, NCH_D, 32), dtype=f8)
    we_h[:, :, 0] = W_e.reshape(NCH_D, P).T.astype(f8)
    bu_h = np.ascontiguousarray(b_u.reshape(NCH_D, P).T).astype(np.float32)
    idf = np.eye(P, dtype=np.float32)
    idb = np.eye(P).astype(bf)

    return [
        {
            "x": np.ascontiguousarray(xs[c]),
            "offs": offs[c],
            "wu": wu_h,
            "wv": wv_h,
            "we": we_h,
            "bu": bu_h,
            "idf": idf,
            "idb": idb,
        }
        for c in range(ncores)
    ]


def kernel(x, last_nodes, W_u, b_u, W_v, W_e, **run_kwargs):
    nc = _get_nc(BSH)
    in_maps = make_in_maps(x, last_nodes, W_u, b_u, W_v, W_e)
    res = run_bass_kernel_spmd(nc, in_maps, core_ids=list(range(NCORES)), **run_kwargs)
    out = np.concatenate([r["out"] for r in res.results], axis=0).astype(np.float32)
    if run_kwargs:
        kernel.last_results = res
    return out



# revision 3
# speedup vs baseline: 13.1414x; 13.1414x over previous
"""AttnReadout kernel for Trainium2 (8 NeuronCores, data-parallel over batch).

Computes, for x:[B,N,D], last_nodes:[B], W_u/W_v:[D,D], b_u:[D], W_e:[D,1]:
    x_u   = x @ W_u + b_u
    x_v   = x[b, last_nodes[b]] @ W_v
    e     = sigmoid(x_u + x_v[:,None]) @ W_e
    alpha = softmax(e, axis=-2)
    out   = sum(x * alpha, axis=-2)          # [B, D]

Sharding: batch is split 8 ways (32 batches per core); the tiny weights are
replicated. No cross-core communication.
"""

import numpy as np
import ml_dtypes
from contextlib import ExitStack

try:
    import concourse.bass as bass
except ImportError:  # stock container: repo lives in /opt
    import sys

    sys.path.insert(0, "/opt/trn_rl_repo")
    import concourse.bass as bass

from concourse import bacc, mybir
import concourse.tile as tile
from concourse.bass_utils import run_bass_kernel_spmd

DT = mybir.dt
BF16 = DT.bfloat16
F32 = DT.float32
F8 = DT.float8e4
DoubleRow = mybir.MatmulPerfMode.DoubleRow

B, N, D = 256, 512, 256
NCORES = 8
BSH = B // NCORES  # 32 batches per core
P = 128
NCH_N = N // P  # 4 chunks of the node axis
NCH_D = D // P  # 2 chunks of the feature axis
GRP = 8  # softmax batching group

Sigmoid = mybir.ActivationFunctionType.Sigmoid
Exp = mybir.ActivationFunctionType.Exp


def build_dma_nc(nb=BSH, reps=1, lgrp=2, dma_eng="gpsimd", dma_dt="f8", grp=GRP):
    """x-load path in isolation: group loads + token consumer + out store."""
    nc = bacc.Bacc("TRN2", target_bir_lowering=False, debug=False, num_devices=NCORES)
    ddt = {"f8": F8, "bf16": BF16, "f32": F32}[dma_dt]
    x_d = nc.dram_tensor("x", [nb, N, D], F32, kind="ExternalInput")
    out_d = nc.dram_tensor("out", [nb, D], F32, kind="ExternalOutput")
    with tile.TileContext(nc) as tc, ExitStack() as ctx:
        consts = ctx.enter_context(tc.tile_pool(name="consts", bufs=1))
        dpool = ctx.enter_context(tc.tile_pool(name="dob", bufs=3))
        zo = consts.tile([nb, D], F32, tag="zo")
        nc.vector.memset(zo[:], 0.0)
        eng = getattr(nc, dma_eng)
        for r in range(reps):
            for g0 in range(0, nb, grp):
                xg = dpool.tile([P, grp, NCH_N, D], ddt, tag="xg")
                for b0 in range(0, grp, lgrp):
                    eng.dma_start(
                        xg[:, b0 : b0 + lgrp],
                        x_d.ap()[g0 + b0 : g0 + b0 + lgrp].rearrange(
                            "b (a p) d -> p b a d", p=P
                        ),
                    )
                nc.vector.tensor_copy(
                    zo[0:1, (g0 // grp) * 16 : (g0 // grp) * 16 + 16],
                    xg[0:1, 0, 0, 0:16],
                )
            nc.sync.dma_start(out_d.ap()[:], zo[:])
    nc.compile()
    return nc


def build_nc(nb=BSH, reps=1, grp=None, pipelined=True, skip_tr=False, skip_cp=False, inter=False, xu_bufs=3, tr_bufs=2, lgrp=2, pool8=True):
    GRP = grp or globals()["GRP"]
    nc = bacc.Bacc("TRN2", target_bir_lowering=False, debug=False, num_devices=NCORES)

    x_d = nc.dram_tensor("x", [nb, N, D], F32, kind="ExternalInput")
    offs_d = nc.dram_tensor("offs", [nb, 1], DT.int32, kind="ExternalInput")
    wu_d = nc.dram_tensor("wu", [P, NCH_D, D], F8, kind="ExternalInput")
    wv_d = nc.dram_tensor("wv", [P, NCH_D, D], BF16, kind="ExternalInput")
    we_d = nc.dram_tensor("we", [P, NCH_D, 32], F8, kind="ExternalInput")
    bu_d = nc.dram_tensor("bu", [P, NCH_D], F32, kind="ExternalInput")
    idf_d = nc.dram_tensor("idf", [P, P], F32, kind="ExternalInput")
    idb_d = nc.dram_tensor("idb", [P, P], BF16, kind="ExternalInput")
    out_d = nc.dram_tensor("out", [nb, D], F32, kind="ExternalOutput")

    with tile.TileContext(nc) as tc, ExitStack() as ctx:
        consts = ctx.enter_context(tc.tile_pool(name="consts", bufs=1))
        # one group-sized x tile per buffer: the whole group loads in a single
        # SWDGE DMA (amortizes the ~1us Q7 descriptor-gen fixed cost 8x)
        xnat = ctx.enter_context(tc.tile_pool(name="xnat", bufs=3))
        xtp = ctx.enter_context(tc.tile_pool(name="xt", bufs=3))
        spool = ctx.enter_context(tc.tile_pool(name="s", bufs=3))
        smallp = ctx.enter_context(tc.tile_pool(name="small", bufs=2))
        # PSUM budget: 3 + 2 + 1 + 1 + 1 = 8 banks exactly.
        pxu = ctx.enter_context(tc.tile_pool(name="pxu", bufs=xu_bufs, space="PSUM"))
        ptr = ctx.enter_context(tc.tile_pool(name="ptr", bufs=tr_bufs, space="PSUM"))
        pe_p = ctx.enter_context(tc.tile_pool(name="pe", bufs=1, space="PSUM"))
        pout = ctx.enter_context(tc.tile_pool(name="pout", bufs=1, space="PSUM"))
        psm = ctx.enter_context(tc.tile_pool(name="psm", bufs=1, space="PSUM"))

        # ---- constants ----
        wu_sb = consts.tile([P, NCH_D, D], F8, tag="wu")
        nc.sync.dma_start(wu_sb[:], wu_d.ap())
        wv_sb = consts.tile([P, NCH_D, D], BF16, tag="wv")
        nc.sync.dma_start(wv_sb[:], wv_d.ap())
        we_sb = consts.tile([P, NCH_D, 32], F8, tag="we")
        nc.sync.dma_start(we_sb[:], we_d.ap())
        bu_sb = consts.tile([P, NCH_D], F32, tag="bu")
        nc.sync.dma_start(bu_sb[:], bu_d.ap())
        idf_sb = consts.tile([P, P], F32, tag="idf")
        nc.sync.dma_start(idf_sb[:], idf_d.ap())
        idb_sb = consts.tile([P, P], BF16, tag="idb")
        nc.sync.dma_start(idb_sb[:], idb_d.ap())
        # fp8 identity for the x transposes (transpose-mode keeps dtype)
        id8_sb = consts.tile([P, P], F8, tag="id8")
        nc.vector.tensor_copy(id8_sb[:], idb_sb[:])
        offs_sb = consts.tile([nb, 1], DT.int32, tag="offs")
        nc.sync.dma_start(offs_sb[:], offs_d.ap())

        # ---- phase 0: gather x_last, compute xvb = W_v^T x_last + b_u ----
        xlast = consts.tile([nb, D], F32, tag="xlast")
        nc.gpsimd.indirect_dma_start(
            out=xlast[:],
            out_offset=None,
            in_=x_d.ap().rearrange("b n d -> (b n) d"),
            in_offset=bass.IndirectOffsetOnAxis(ap=offs_sb[:, :1], axis=0),
        )
        # transpose to [D, nb] (d-major) so the W_v matmul can contract over d
        xlt_ps = psm.tile([P, NCH_D, nb], F32, tag="sm")
        for c in range(NCH_D):
            nc.tensor.matmul(
                xlt_ps[:, c, :], xlast[:, c * P : (c + 1) * P], idf_sb[:nb, :nb]
            )
        xlt = consts.tile([P, NCH_D, nb], BF16, tag="xlt")
        nc.vector.tensor_copy(xlt[:], xlt_ps[:])

        xt_const = None
        if skip_tr or skip_cp:  # timing experiments
            xt_const = consts.tile([P, 2 * N], F8, tag="xtc")
            nc.vector.memset(xt_const[:], 0.001)

        xvb = consts.tile([P, NCH_D, nb], F32, tag="xvb")
        for j in range(NCH_D):
            xv_ps = psm.tile([P, nb], F32, tag="sm")
            for c in range(NCH_D):
                nc.tensor.matmul(
                    xv_ps[:],
                    wv_sb[:, c, j * P : (j + 1) * P],
                    xlt[:, c, :],
                    start=(c == 0),
                    stop=(c == NCH_D - 1),
                )
            nc.vector.tensor_copy(xvb[:, j, :], xv_ps[:])
            nc.vector.tensor_scalar_add(xvb[:, j, :], xvb[:, j, :], bu_sb[:, j : j + 1])

        # ---- main loop ----
        # The "final" phase (softmax + weighted pooling) of each group is
        # emitted one group late: its softmax/p-transpose prologue goes in
        # front of the next group's batch loop, and its per-batch pooling
        # matvecs are interleaved between the next group's e-matvecs on
        # DISJOINT PE column groups, so the hardware runs them concurrently.

        def final_prologue(g, gb, e_all):
            # softmax over n (batched across the group)
            m_t = smallp.tile([gb, 1], F32, tag="mx")
            nc.vector.tensor_reduce(
                m_t[:], e_all[:], axis=mybir.AxisListType.X, op=mybir.AluOpType.max
            )
            mn_t = smallp.tile([gb, 1], F32, tag="mn")
            nc.vector.tensor_scalar_mul(mn_t[:], m_t[:], -1.0)
            p_t = smallp.tile([gb, N], BF16, tag="p")
            sum_t = smallp.tile([gb, 1], F32, tag="sum")
            nc.scalar.activation(
                p_t[:], e_all[:], Exp, bias=mn_t[:, :1], accum_out=sum_t[:, :1]
            )
            r_t = smallp.tile([gb, 1], F32, tag="r")
            nc.vector.reciprocal(r_t[:], sum_t[:])

            # transpose p to n-major for use as matmul weights
            pt_ps = psm.tile([P, NCH_N, gb], F32, tag="sm")
            for i in range(NCH_N):
                nc.tensor.matmul(
                    pt_ps[:, i, :], p_t[:, i * P : (i + 1) * P], idb_sb[:gb, :gb]
                )
            # gb real columns + 32 zero columns so each batch's lhsT slab
            # [bb : bb+32] is in-bounds (col 0 real, rest harmless)
            pt_sb = smallp.tile([P, NCH_N, gb + 32], BF16, tag="pt")
            nc.vector.memset(pt_sb[:, :, gb : gb + 32], 0.0)
            nc.vector.tensor_copy(pt_sb[:, :, 0:gb], pt_ps[:])
            outall = smallp.tile([gb, D], F32, tag="oall")
            return {"g": g, "gb": gb, "pt": pt_sb, "r": r_t, "outall": outall,
                    "o_ps": None, "xbs": None}

        def final_mms(ctx, bb, js):
            # pooling matvecs for prev-group batch bb, n-chunks in `js`;
            # column group (bb+2)%4 — disjoint from the current e-matvec's
            qo = (bb + 2) % 4
            xg_, bi = ctx["xbs"][bb]
            for i in js:
                nc.tensor.matmul(
                    ctx["o_ps"][qo * 32 : qo * 32 + 32, :],
                    ctx["pt"][:, i, bb : bb + 32],
                    xg_[:, bi, i, :],
                    start=(i == 0),
                    stop=(i == NCH_N - 1),
                    tile_position=(0, qo * 32),
                )

        def final_drain(ctx, bb):
            # after each quad: PSUM -> SBUF, then permuted row gather
            # (row base+m sits at partition 32*((m+2)%4))
            o_sb = spool.tile([P, D], F32, tag="osb")
            nc.vector.tensor_copy(o_sb[:], ctx["o_ps"][:])
            base = bb - (bb % 4)
            nc.sync.dma_start(
                ctx["outall"][base : base + 2, :], o_sb[64 : 97 : 32, :]
            )
            nc.sync.dma_start(
                ctx["outall"][base + 2 : base + 4, :], o_sb[0 : 33 : 32, :]
            )

        def final_epilogue(ctx):
            outall, gb = ctx["outall"], ctx["gb"]
            nc.vector.tensor_scalar_mul(outall[:], outall[:], ctx["r"][:, :1])
            g0 = (ctx["g"] % nb_grp) * GRP
            nc.sync.dma_start(out_d.ap()[g0 : g0 + gb, :], outall[:])

        def compute_group(g, prev):
            gb = min(GRP, nb - (g % nb_grp) * GRP)
            assert gb % 4 == 0
            ctx = None
            if prev is not None:
                pg, pgb, pe_all, pxbs = prev
                ctx = final_prologue(pg, pgb, pe_all)
                ctx["xbs"] = pxbs
            e_all = smallp.tile([gb, N], F32, tag="eall")
            g0 = (g % nb_grp) * GRP
            # load the group's x in gb/lgrp SWDGE DMAs (f32 -> fp8/bf16 cast),
            # natural layout per batch: xg[p, bb, a, d] = x[g0+bb, a*128+p, d].
            # lgrp batches per DMA amortizes the ~1us Q7 descriptor-gen fixed
            # cost while keeping completion granularity fine enough to feed
            # the first transposes early.
            xg = xnat.tile([P, gb, NCH_N, D], F8, tag="xg")
            for b0 in range(0, gb, lgrp):
                b1 = min(b0 + lgrp, gb)
                nc.gpsimd.dma_start(
                    xg[:, b0:b1],
                    x_d.ap()[g0 + b0 : g0 + b1].rearrange(
                        "b (a p) d -> p b a d", p=P
                    ),
                )
            xbs = []
            e_ps = None
            for bb in range(gb):
                b = g0 + bb
                xbs.append((xg, bb))

                # transpose to d-major XT [d, n] via PE (transpose-mode,
                # bf16 PSUM). All 8 blocks land in ONE full-bank PSUM tile,
                # drained by ONE bit-copy per batch (alternating ACT/DVE) —
                # per-block drains ping-pong with PE on bank hazards.
                if skip_tr:
                    xt = xt_const
                else:
                    # PAIR transpose: view fp8 x as bf16 pairs [128, 128] per
                    # n-chunk — 4 transposes/batch instead of 8. Result layout
                    # xt[i, k, t] = x[b, k, 2i+t] (d pair-interleaved), which
                    # matches DoubleRow's (Ki, Ko) pairing with host-paired Wu.
                    xt = xtp.tile([P, 2 * N], F8, tag="xt")
                    tr = ptr.tile([P, NCH_N, P], BF16, tag="tr")
                    for a in range(NCH_N):
                        nc.tensor.transpose(
                            tr[:, a, :],
                            xg[:, bb, a, :].bitcast(BF16),
                            idb_sb[:],
                        )
                    if skip_cp:
                        xt = xt_const
                    else:
                        nc.vector.tensor_copy(xt[:].bitcast(F32), tr[:].bitcast(F32))

                # x_u^T = W_u^T @ XT: fp8 DoubleRow packs the full d=256
                # contraction into one matmul per output chunk j. Wu is
                # host-paired (d = 2*Ki + Ko) to match xt's pair interleave.
                s_t = spool.tile([P, NCH_D, N], F8, tag="s")
                for j in range(NCH_D):
                    xu = pxu.tile([P, N], F32, tag="xu")
                    nc.tensor.matmul(
                        xu[:],
                        wu_sb[:, :, j * P : (j + 1) * P],
                        xt[:].rearrange("p (k t) -> p t k", t=2),
                        perf_mode=DoubleRow,
                    )
                    nc.scalar.activation(
                        s_t[:, j, :], xu[:], Sigmoid, bias=xvb[:, j, b : b + 1]
                    )

                # e[n] = W_e^T @ S : one fp8 DoubleRow matvec (M=32 zero-pad
                # trick unchanged); lands on psum partition 32*(bb%4).
                # Interleaved with the previous group's pooling matvecs
                # (disjoint column groups -> concurrent on the PE array).
                q = bb % 4
                if q == 0:
                    e_ps = pe_p.tile([P, N], F32, tag="e")
                    if ctx is not None and bb < ctx["gb"]:
                        ctx["o_ps"] = pout.tile([P, D], F32, name="o_ps", tag="o")
                # (plain fp8 matmuls here: DoubleRow + col tile_position is
                # illegal — XBUS budget)
                for j in range(NCH_D):
                    nc.tensor.matmul(
                        e_ps[q * 32 : q * 32 + 32, :],
                        we_sb[:, j, :],
                        s_t[:, j, :],
                        start=(j == 0),
                        stop=(j == NCH_D - 1),
                        tile_position=(0, q * 32),
                    )
                    if ctx is not None and bb < ctx["gb"]:
                        final_mms(ctx, bb, (2 * j, 2 * j + 1))
                if ctx is not None and bb < ctx["gb"] and (q == 3 or bb == ctx["gb"] - 1):
                    final_drain(ctx, bb)
                if q == 3 or bb == gb - 1:
                    # engines can't address strided partitions, but DMA can:
                    # PSUM -> SBUF copy (contiguous), then SBUF->SBUF DMA gather
                    e_sb = spool.tile([P, N], F32, tag="esb")
                    nc.vector.tensor_copy(e_sb[:], e_ps[:])
                    nc.sync.dma_start(
                        e_all[bb - q : bb + 1, :], e_sb[0 : 32 * q + 1 : 32, :]
                    )
            if ctx is not None:
                for bb in range(gb, ctx["gb"]):  # leftover when prev group bigger
                    q = bb % 4
                    if q == 0:
                        ctx["o_ps"] = pout.tile([P, D], F32, name="o_ps", tag="o")
                    final_mms(ctx, bb, range(NCH_N))
                    if q == 3 or bb == ctx["gb"] - 1:
                        final_drain(ctx, bb)
                final_epilogue(ctx)
            return gb, e_all, xbs

        def final_tail(prev):
            # the last group's final phase has no next group to hide in
            pg, pgb, pe_all, pxbs = prev
            ctx = final_prologue(pg, pgb, pe_all)
            ctx["xbs"] = pxbs
            for bb in range(pgb):
                q = bb % 4
                if q == 0:
                    ctx["o_ps"] = pout.tile([P, D], F32, name="o_ps", tag="o")
                final_mms(ctx, bb, range(NCH_N))
                if q == 3 or bb == pgb - 1:
                    final_drain(ctx, bb)
            final_epilogue(ctx)

        # reps>1 repeats the whole sweep (for slope-based timing)
        nb_grp = (nb + GRP - 1) // GRP
        pending = None
        for g in range(nb_grp * reps):
            if inter:
                pending = (g,) + compute_group(g, pending)
            else:
                done = (g,) + compute_group(g, None)
                if pending is not None:
                    final_tail(pending)
                pending = done
        final_tail(pending)

    nc.compile()
    return nc


_NC_CACHE = {}


def _get_nc(nb=BSH, reps=1):
    if (nb, reps) not in _NC_CACHE:
        _NC_CACHE[(nb, reps)] = build_nc(nb, reps)
    return _NC_CACHE[(nb, reps)]


def make_in_maps(x, last_nodes, W_u, b_u, W_v, W_e, ncores=NCORES):
    x = np.asarray(x, dtype=np.float32)
    last_nodes = np.asarray(last_nodes).astype(np.int64)
    W_u = np.asarray(W_u, dtype=np.float32)
    b_u = np.asarray(b_u, dtype=np.float32)
    W_v = np.asarray(W_v, dtype=np.float32)
    W_e = np.asarray(W_e, dtype=np.float32)

    nb = x.shape[0] // ncores
    xs = x.reshape(ncores, nb, N, D)
    ln = last_nodes.reshape(ncores, nb)
    offs = (np.arange(nb)[None, :] * N + ln).astype(np.int32).reshape(ncores, nb, 1)

    bf = ml_dtypes.bfloat16
    f8 = ml_dtypes.float8_e4m3
    # DoubleRow pairing: wu_h[i, t, m] = W_u[2i+t, m] matches the
    # pair-interleaved x^T layout from the bf16-view transposes.
    wu_h = np.ascontiguousarray(W_u.reshape(P, 2, D)).astype(f8)
    wv_h = np.ascontiguousarray(
        W_v.reshape(NCH_D, P, D).transpose(1, 0, 2)
    ).astype(bf)
    we_h = np.zeros((P, NCH_D, 32), dtype=f8)
    we_h[:, :, 0] = W_e.reshape(NCH_D, P).T.astype(f8)
    bu_h = np.ascontiguousarray(b_u.reshape(NCH_D, P).T).astype(np.float32)
    idf = np.eye(P, dtype=np.float32)
    idb = np.eye(P).astype(bf)

    return [
        {
            "x": np.ascontiguousarray(xs[c]),
            "offs": offs[c],
            "wu": wu_h,
            "wv": wv_h,
            "we": we_h,
            "bu": bu_h,
            "idf": idf,
            "idb": idb,
        }
        for c in range(ncores)
    ]


def kernel(x, last_nodes, W_u, b_u, W_v, W_e, **run_kwargs):
    nc = _get_nc(BSH)
    in_maps = make_in_maps(x, last_nodes, W_u, b_u, W_v, W_e)
    res = run_bass_kernel_spmd(nc, in_maps, core_ids=list(range(NCORES)), **run_kwargs)
    out = np.concatenate([r["out"] for r in res.results], axis=0).astype(np.float32)
    if run_kwargs:
        kernel.last_results = res
    return out

